# revision 4
# baseline (speedup 1.0000x reference)
"""Trainium2 Bass kernel for nn_Decoder: Bahdanau attention + LSTM step + vocab projection.

Sharding (8 NeuronCores, single NEFF with AllGather collectives):
  - attention: data-parallel over batch (8 batches/core), encoder_states read once
  - LSTM: tensor-parallel over the hidden dim (128 h-rows/core), x_in AllGathered
  - fc projection: tensor-parallel column split of [H, V] (6400 padded V-cols/core),
    h_next AllGathered

All heavy matmuls run as float32r (single-pass fp32 on the PE); energy dot products
run on the vector engine via scalar_tensor_tensor with free-dim accumulation.
"""

import sys

if "/opt/trn_rl_repo" not in sys.path:
    sys.path.insert(0, "/opt/trn_rl_repo")

import numpy as np

import concourse.bass as bass
import concourse.tile as tile
from concourse import bacc, mybir
from concourse.bass_utils import run_bass_kernel_spmd
from concourse.masks import make_identity

# problem dims
B, S, H, E, V = 64, 1024, 1024, 512, 50257
NCORES = 8
BC = B // NCORES          # local batches per core
NT = S // 128             # s-tiles per batch
D2H = 2 * H               # 2048
DIN = D2H + E             # 2560, x_in width
KT_IH = DIN // 128        # 20
KT_HH = H // 128          # 8
VC = 6400                 # padded V-cols per core (8*6400 = 51200 >= V)
NTV = 256                 # v-tile width in the projection
NV = VC // NTV            # 25 v-tiles

f32 = mybir.dt.float32
f32r = mybir.dt.float32r

_COMPILED = None


class _StopBuild(Exception):
    def __init__(self, nc):
        self.nc = nc


def _build():
    import os
    stop_at = int(os.environ.get("STOP_AT", "99"))
    nc = bacc.Bacc("TRN2", target_bir_lowering=False, debug=False, num_devices=NCORES)

    es_d = nc.dram_tensor("es_k", [BC, S, D2H], f32, kind="ExternalInput")
    weE_d = nc.dram_tensor("weE_rep", [128, D2H], f32, kind="ExternalInput")
    weH_d = nc.dram_tensor("weH_pc", [128, KT_HH], f32, kind="ExternalInput")
    be_d = nc.dram_tensor("be_in", [1, 1], f32, kind="ExternalInput")
    hTl_d = nc.dram_tensor("hT_loc", [H, BC], f32, kind="ExternalInput")
    hTf_d = nc.dram_tensor("hT_full", [H, B], f32, kind="ExternalInput")
    cTk_d = nc.dram_tensor("cT_k", [128, B], f32, kind="ExternalInput")
    xemb_d = nc.dram_tensor("xemb_k", [BC, E], f32, kind="ExternalInput")
    wih_d = nc.dram_tensor("wih_k", [KT_IH, 128, 512], f32, kind="ExternalInput")
    whh_d = nc.dram_tensor("whh_k", [KT_HH, 128, 512], f32, kind="ExternalInput")
    bias_d = nc.dram_tensor("bias_k", [128, 4], f32, kind="ExternalInput")
    wf_d = nc.dram_tensor("wf_k", [NV, KT_HH, 128, NTV], f32, kind="ExternalInput")
    bfr_d = nc.dram_tensor("bfr_k", [1, VC], f32, kind="ExternalInput")

    hT_o = nc.dram_tensor("hT_out", [128, B], f32, kind="ExternalOutput")
    cT_o = nc.dram_tensor("cT_out", [128, B], f32, kind="ExternalOutput")
    preds_o = nc.dram_tensor("preds_k", [B, VC], f32, kind="ExternalOutput")

    AF = mybir.ActivationFunctionType
    OP = mybir.AluOpType
    groups = [list(range(NCORES))]

    with tile.TileContext(nc) as tc:
        with (
            tc.tile_pool(name="dram", bufs=1, space="DRAM") as dram,
            tc.tile_pool(name="const", bufs=1) as const_pool,
            tc.tile_pool(name="small", bufs=1) as small,
        ):
            # DRAM bounce buffers for collectives
            bx_in = dram.tile([BC, DIN], f32)
            bx_out = dram.tile([B, DIN], f32)
            bh_in = dram.tile([128, B], f32)
            bh_out = dram.tile([H, B], f32)

            # constants
            weE = const_pool.tile([128, D2H], f32)
            nc.sync.dma_start(weE[:], weE_d.ap())
            weH = const_pool.tile([128, KT_HH], f32)
            nc.sync.dma_start(weH[:], weH_d.ap())
            be_sb = const_pool.tile([1, 1], f32)
            nc.sync.dma_start(be_sb[:], be_d.ap())
            ones = const_pool.tile([128, 1], f32)
            nc.vector.memset(ones[:], 1.0)
            ones64 = const_pool.tile([1, B], f32)
            nc.vector.memset(ones64[:], 1.0)
            ident = const_pool.tile([128, 128], f32)
            make_identity(nc, ident[:])

            # ---- per-batch energy offset: off[b] = hidden[b] . We_h + be ----
            hTl_r = small.tile([128, BC, KT_HH], f32)
            nc.sync.dma_start(
                hTl_r[:], hTl_d.ap().rearrange("(c p) b -> p b c", p=128)
            )
            htmp = small.tile([128, BC, KT_HH], f32)
            nc.vector.tensor_tensor(
                out=htmp[:],
                in0=hTl_r[:],
                in1=weH[:][:, None, :].to_broadcast([128, BC, KT_HH]),
                op=OP.mult,
            )
            hpart = small.tile([128, BC], f32)
            nc.vector.tensor_reduce(
                out=hpart[:], in_=htmp[:], axis=mybir.AxisListType.X, op=OP.add
            )

            with tc.tile_pool(name="psum_off", bufs=1, space="PSUM") as psum_off:
                off_ps = psum_off.tile([1, BC], f32, space="PSUM")
                nc.tensor.matmul(
                    off_ps[:], lhsT=ones[:], rhs=hpart[:], start=True, stop=True
                )
                off_sb = small.tile([1, BC], f32)
                nc.scalar.activation(
                    off_sb[:], off_ps[:], AF.Identity, bias=be_sb[:], scale=1.0
                )
            off_rep = small.tile([128, BC], f32)
            nc.gpsimd.partition_broadcast(off_rep[:], off_sb[:])

            # ---- attention ----
            with (
                tc.tile_pool(name="es", bufs=12) as es_pool,
                tc.tile_pool(name="scr", bufs=2) as scratch,
                tc.tile_pool(name="sm3", bufs=3) as sm3,
                tc.tile_pool(name="psU", bufs=1, space="PSUM") as psum_u,
                tc.tile_pool(name="psS", bufs=2, space="PSUM") as psum_s,
            ):
                for b in range(BC):
                    ets = []
                    A_b = sm3.tile([128, NT], f32, tag="A")
                    for t in range(NT):
                        et = es_pool.tile([128, D2H], f32r, tag="es")
                        nc.gpsimd.dma_start(
                            et[:], es_d.ap()[b, t * 128:(t + 1) * 128, :]
                        )
                        ets.append(et)
                        prod = scratch.tile([128, D2H], f32, tag="prod")
                        nc.vector.scalar_tensor_tensor(
                            out=prod[:],
                            in0=et[:].bitcast(f32),
                            scalar=1.0,
                            in1=weE[:],
                            op0=OP.mult,
                            op1=OP.mult,
                            accum_out=A_b[:, t:t + 1],
                        )
                    # softmax pieces for batch b (no max-subtraction needed:
                    # relu'd energies are in [0, ~30], exp is safe in f32)
                    R = scratch.tile([128, NT], f32, tag="R")
                    nc.scalar.activation(
                        R[:], A_b[:], AF.Relu,
                        bias=off_rep[:, b:b + 1], scale=1.0,
                    )
                    Eexp_b = sm3.tile([128, NT], f32r, tag="E")
                    Zp_b = sm3.tile([128, 1], f32, tag="Zp")
                    nc.scalar.activation(
                        Eexp_b[:], R[:], AF.Exp, accum_out=Zp_b[:]
                    )
                    zps = psum_s.tile([1, 1], f32, space="PSUM", tag="zt")
                    nc.tensor.matmul(
                        zps[:], lhsT=ones[:], rhs=Zp_b[:], start=True, stop=True
                    )
                    rZ_b = sm3.tile([1, 1], f32, tag="rz")
                    nc.vector.reciprocal(rZ_b[:], zps[:])

                    # unnormalized context row U = sum_s exp(e_s) * es[s, :]
                    U = psum_u.tile([1, D2H], f32, space="PSUM", tag="U")
                    for t in range(NT):
                        for j in range(D2H // 512):
                            nc.tensor.matmul(
                                U[:, j * 512:(j + 1) * 512],
                                lhsT=Eexp_b[:, t:t + 1],
                                rhs=ets[t][:, j * 512:(j + 1) * 512],
                                start=(t == 0),
                                stop=(t == NT - 1),
                            )
                    stg = scratch.tile([1, D2H], f32, tag="stage")
                    nc.scalar.activation(
                        stg[:], U[:], AF.Copy, scale=rZ_b[:]
                    )
                    nc.sync.dma_start(bx_in[b:b + 1, :D2H], stg[:])

                # embedded token -> x_in bounce tail
                xemb_sb = sm3.tile([BC, E], f32, tag="xe")
                nc.sync.dma_start(xemb_sb[:], xemb_d.ap())
                nc.sync.dma_start(bx_in[:, D2H:], xemb_sb[:])

            # ---- gather x_in across cores ----
            if stop_at < 2:
                raise _StopBuild(nc)
            nc.gpsimd.collective_compute(
                "AllGather", OP.bypass, replica_groups=groups,
                ins=[bx_in.opt()], outs=[bx_out.opt()],
            )

            # ---- LSTM (tensor-parallel over 128 h-rows) ----
            if stop_at < 3:
                raise _StopBuild(nc)
            with (
                tc.tile_pool(name="lstm", bufs=1) as lp,
                tc.tile_pool(name="psT", bufs=2, space="PSUM") as psum_t,
                tc.tile_pool(name="psG", bufs=4, space="PSUM") as psum_g,
            ):
                x_full = lp.tile([B, DIN], f32)
                nc.sync.dma_start(x_full[:], bx_out[:])
                x_inT = lp.tile([128, KT_IH, B], f32)
                for c in range(KT_IH):
                    tp = psum_t.tile([128, B], f32, space="PSUM", tag="tp")
                    nc.tensor.transpose(
                        tp[:], x_full[:, c * 128:(c + 1) * 128], ident[:B, :B]
                    )
                    nc.vector.tensor_copy(x_inT[:, c, :], tp[:])

                hTf_sb = lp.tile([128, KT_HH, B], f32)
                nc.sync.dma_start(
                    hTf_sb[:], hTf_d.ap().rearrange("(t p) b -> p t b", p=128)
                )
                wih_sb = lp.tile([128, KT_IH, 512], f32)
                nc.sync.dma_start(
                    wih_sb[:], wih_d.ap().rearrange("t p g -> p t g")
                )
                whh_sb = lp.tile([128, KT_HH, 512], f32)
                nc.sync.dma_start(
                    whh_sb[:], whh_d.ap().rearrange("t p g -> p t g")
                )
                bias_sb = lp.tile([128, 4], f32)
                nc.sync.dma_start(bias_sb[:], bias_d.ap())
                cT_sb = lp.tile([128, B], f32)
                nc.sync.dma_start(cT_sb[:], cTk_d.ap())

                gps = []
                for c in range(4):
                    gp = psum_g.tile([128, B], f32, space="PSUM", tag="g")
                    gps.append(gp)
                    for kt in range(KT_IH):
                        nc.tensor.matmul(
                            gp[:],
                            lhsT=wih_sb[:, kt, c * 128:(c + 1) * 128],
                            rhs=x_inT[:, kt, :],
                            start=(kt == 0),
                            stop=False,
                        )
                    for kt in range(KT_HH):
                        nc.tensor.matmul(
                            gp[:],
                            lhsT=whh_sb[:, kt, c * 128:(c + 1) * 128],
                            rhs=hTf_sb[:, kt, :],
                            start=False,
                            stop=(kt == KT_HH - 1),
                        )

                # gates: order i, f, g, o
                acts = []
                for c, fn in enumerate([AF.Sigmoid, AF.Sigmoid, AF.Tanh, AF.Sigmoid]):
                    av = lp.tile([128, B], f32, tag=f"act{c}")
                    nc.scalar.activation(
                        av[:], gps[c][:], fn, bias=bias_sb[:, c:c + 1], scale=1.0
                    )
                    acts.append(av)
                i_s, f_s, g_t, o_s = acts

                t1 = lp.tile([128, B], f32)
                nc.vector.tensor_tensor(out=t1[:], in0=f_s[:], in1=cT_sb[:], op=OP.mult)
                t2 = lp.tile([128, B], f32)
                nc.vector.tensor_tensor(out=t2[:], in0=i_s[:], in1=g_t[:], op=OP.mult)
                cT_new = lp.tile([128, B], f32)
                nc.vector.tensor_tensor(out=cT_new[:], in0=t1[:], in1=t2[:], op=OP.add)
                th = lp.tile([128, B], f32)
                nc.scalar.activation(th[:], cT_new[:], AF.Tanh)
                hT_new = lp.tile([128, B], f32)
                nc.vector.tensor_tensor(out=hT_new[:], in0=o_s[:], in1=th[:], op=OP.mult)

                nc.sync.dma_start(hT_o.ap(), hT_new[:])
                nc.sync.dma_start(cT_o.ap(), cT_new[:])
                nc.sync.dma_start(bh_in[:], hT_new[:])

            # ---- gather h_next across cores ----
            if stop_at < 4:
                raise _StopBuild(nc)
            nc.gpsimd.collective_compute(
                "AllGather", OP.bypass, replica_groups=groups,
                ins=[bh_in.opt()], outs=[bh_out.opt()],
            )

            # ---- vocab projection (tensor-parallel over V) ----
            if stop_at < 5:
                raise _StopBuild(nc)
            with (
                tc.tile_pool(name="proj", bufs=1) as pp,
                tc.tile_pool(name="wf", bufs=3) as wfp,
                tc.tile_pool(name="psP", bufs=2, space="PSUM") as psum_p,
            ):
                hTn_sb = pp.tile([128, KT_HH, B], f32)
                nc.sync.dma_start(
                    hTn_sb[:], bh_out[:].rearrange("(t p) b -> p t b", p=128)
                )
                bfr_sb = pp.tile([1, VC], f32)
                nc.sync.dma_start(bfr_sb[:], bfr_d.ap())
                preds_sb = pp.tile([B, VC], f32)

                for vt in range(NV):
                    wf_sb = wfp.tile([128, KT_HH, NTV], f32, tag="wf")
                    nc.sync.dma_start(
                        wf_sb[:], wf_d.ap()[vt].rearrange("t p g -> p t g")
                    )
                    pp_ps = psum_p.tile([B, NTV], f32, space="PSUM", tag="pp")
                    for kt in range(KT_HH):
                        nc.tensor.matmul(
                            pp_ps[:],
                            lhsT=hTn_sb[:, kt, :],
                            rhs=wf_sb[:, kt, :],
                            start=(kt == 0),
                            stop=False,
                        )
                    nc.tensor.matmul(
                        pp_ps[:],
                        lhsT=ones64[:],
                        rhs=bfr_sb[:, vt * NTV:(vt + 1) * NTV],
                        start=False,
                        stop=True,
                    )
                    nc.scalar.copy(preds_sb[:, vt * NTV:(vt + 1) * NTV], pp_ps[:])

                nc.sync.dma_start(preds_o.ap(), preds_sb[:])

    nc.compile()
    return nc


def _prep_inputs(encoder_states, x_tok, hidden, cell, emb, We, be,
                 W_ih, W_hh, b_ih, b_hh, Wf, bf):
    es = np.ascontiguousarray(encoder_states, dtype=np.float32)
    hidden = np.asarray(hidden, dtype=np.float32)
    cell = np.asarray(cell, dtype=np.float32)
    We = np.asarray(We, dtype=np.float32)
    x_tok = np.asarray(x_tok)

    hT = np.ascontiguousarray(hidden.T)                      # [H, B]
    cT = np.ascontiguousarray(cell.T)                        # [H, B]
    xemb = np.asarray(emb, dtype=np.float32)[x_tok[:, 0]]    # [B, E]
    weE_rep = np.ascontiguousarray(
        np.broadcast_to(We[0, :D2H][None, :], (128, D2H))
    )
    weH_pc = np.ascontiguousarray(We[0, D2H:].reshape(KT_HH, 128).T)  # [128, 8]
    be_in = np.asarray(be, dtype=np.float32).reshape(1, 1)

    W_ihT = np.ascontiguousarray(np.asarray(W_ih, dtype=np.float32).T)  # [DIN, 4H]
    W_hhT = np.ascontiguousarray(np.asarray(W_hh, dtype=np.float32).T)  # [H, 4H]
    bsum = np.asarray(b_ih, dtype=np.float32) + np.asarray(b_hh, dtype=np.float32)

    WfT = np.zeros((H, NCORES * VC), np.float32)
    WfT[:, :V] = np.asarray(Wf, dtype=np.float32).T
    bf_pad = np.zeros(NCORES * VC, np.float32)
    bf_pad[:V] = np.asarray(bf, dtype=np.float32)

    in_maps = []
    for k in range(NCORES):
        bs = slice(k * BC, (k + 1) * BC)
        hs = slice(k * 128, (k + 1) * 128)
        # gate columns for this core's h-slice: i, f, g, o
        gcols = np.concatenate([np.arange(g * H + k * 128, g * H + k * 128 + 128)
                                for g in range(4)])
        wih_k = np.ascontiguousarray(
            W_ihT[:, gcols].reshape(KT_IH, 128, 512)
        )
        whh_k = np.ascontiguousarray(
            W_hhT[:, gcols].reshape(KT_HH, 128, 512)
        )
        bias_k = np.ascontiguousarray(bsum[gcols].reshape(4, 128).T)  # [128, 4]
        wf_k = np.ascontiguousarray(
            WfT[:, k * VC:(k + 1) * VC]
            .reshape(KT_HH, 128, NV, NTV)
            .transpose(2, 0, 1, 3)
        )
        in_maps.append({
            "es_k": np.ascontiguousarray(es[bs]),
            "weE_rep": weE_rep,
            "weH_pc": weH_pc,
            "be_in": be_in,
            "hT_loc": np.ascontiguousarray(hT[:, bs]),
            "hT_full": hT,
            "cT_k": np.ascontiguousarray(cT[hs, :]),
            "xemb_k": np.ascontiguousarray(xemb[bs]),
            "wih_k": wih_k,
            "whh_k": whh_k,
            "bias_k": bias_k,
            "wf_k": wf_k,
            "bfr_k": bf_pad[None, k * VC:(k + 1) * VC],
        })
    return in_maps


def kernel(encoder_states, x_tok, hidden, cell, emb, We, be,
           W_ih, W_hh, b_ih, b_hh, Wf, bf, _trace=False):
    global _COMPILED
    if _COMPILED is None:
        _COMPILED = _build()
    nc = _COMPILED

    in_maps = _prep_inputs(encoder_states, x_tok, hidden, cell, emb, We, be,
                           W_ih, W_hh, b_ih, b_hh, Wf, bf)
    res = run_bass_kernel_spmd(
        nc, in_maps, core_ids=list(range(NCORES)), trace=_trace
    )
    outs = res.results

    preds = np.concatenate([outs[k]["preds_k"] for k in range(NCORES)], axis=1)
    preds = preds[:, :V][:, None, :]                      # [B, 1, V]
    h_next = np.concatenate(
        [outs[k]["hT_out"].T for k in range(NCORES)], axis=1
    )                                                     # [B, H]
    c_next = np.concatenate(
        [outs[k]["cT_out"].T for k in range(NCORES)], axis=1
    )
    if _trace:
        return (preds, h_next, c_next), res
    return preds, h_next, c_next


# revision 14
# speedup vs baseline: 430.3741x; 430.3741x over previous
"""Trainium2 Bass kernel for nn_Decoder: Bahdanau attention + LSTM step + vocab projection.

Sharding (8 NeuronCores, single NEFF with AllGather collectives):
  - attention: data-parallel over batch (8 batches/core), encoder_states read once
  - LSTM: tensor-parallel over the hidden dim (128 h-rows/core), x_in AllGathered
  - fc projection: tensor-parallel column split of [H, V] (6400 padded V-cols/core),
    h_next AllGathered

All heavy matmuls run as float32r (single-pass fp32 on the PE); energy dot products
run on the vector engine via scalar_tensor_tensor with free-dim accumulation.
"""

import sys

if "/opt/trn_rl_repo" not in sys.path:
    sys.path.insert(0, "/opt/trn_rl_repo")

import ml_dtypes
import numpy as np

import concourse.bass as bass
import concourse.tile as tile
from concourse import bacc, mybir
from concourse.bass_utils import run_bass_kernel_spmd
from concourse.masks import make_identity

# problem dims
B, S, H, E, V = 64, 1024, 1024, 512, 50257
NCORES = 8
BC = B // NCORES          # local batches per core
NT = S // 128             # s-tiles per batch
D2H = 2 * H               # 2048
DIN = D2H + E             # 2560, x_in width
KT_IH = DIN // 128        # 20
KT_HH = H // 128          # 8
VC = 6400                 # padded V-cols per core (8*6400 = 51200 >= V)
NTV = 256                 # v-tile width in the projection
NV = VC // NTV            # 25 v-tiles

f32 = mybir.dt.float32
f32r = mybir.dt.float32r
bf16 = mybir.dt.bfloat16
f16 = mybir.dt.float16

_COMPILED = None


class _StopBuild(Exception):
    def __init__(self, nc):
        self.nc = nc


def _build():
    import os
    stop_at = int(os.environ.get("STOP_AT", "99"))
    nc = bacc.Bacc("TRN2", target_bir_lowering=False, debug=False, num_devices=NCORES)

    es_d = nc.dram_tensor("es_k", [BC, S, D2H], f16, kind="ExternalInput")
    weE_d = nc.dram_tensor("weE_rep", [128, D2H], f16, kind="ExternalInput")
    weH_d = nc.dram_tensor("weH_pc", [128, KT_HH], f32, kind="ExternalInput")
    be_d = nc.dram_tensor("be_in", [1, 1], f32, kind="ExternalInput")
    hTl_d = nc.dram_tensor("hT_loc", [H, BC], f32, kind="ExternalInput")
    hTf_d = nc.dram_tensor("hT_full", [H, B], f16, kind="ExternalInput")
    cTk_d = nc.dram_tensor("cT_k", [128, B], f32, kind="ExternalInput")
    xembT_d = nc.dram_tensor("xembT", [E, B], f16, kind="ExternalInput")
    wih_d = nc.dram_tensor("wih_k", [KT_IH, 128, 512], f16, kind="ExternalInput")
    whh_d = nc.dram_tensor("whh_k", [KT_HH, 128, 512], f16, kind="ExternalInput")
    bias_d = nc.dram_tensor("bias_k", [128, 4], f32, kind="ExternalInput")
    wf_d = nc.dram_tensor("wf_k", [NV, KT_HH, 128, NTV], f16, kind="ExternalInput")
    bfr_d = nc.dram_tensor("bfr_k", [1, VC], f16, kind="ExternalInput")

    hT_o = nc.dram_tensor("hT_out", [128, B], f32, kind="ExternalOutput")
    cT_o = nc.dram_tensor("cT_out", [128, B], f32, kind="ExternalOutput")
    preds_o = nc.dram_tensor("preds_k", [B, VC], f32, kind="ExternalOutput")

    AF = mybir.ActivationFunctionType
    OP = mybir.AluOpType
    groups = [list(range(NCORES))]

    with tile.TileContext(nc) as tc:
        with (
            tc.tile_pool(name="dram", bufs=1, space="DRAM") as dram,
            tc.tile_pool(name="const", bufs=1) as const_pool,
            tc.tile_pool(name="small", bufs=1) as small,
        ):
            # DRAM bounce buffers for collectives
            bx_in_a = dram.tile([BC // 2, D2H], f16)
            bx_in_b = dram.tile([BC // 2, D2H], f16)
            bx_out_a = dram.tile([B // 2, D2H], f16)
            bx_out_b = dram.tile([B // 2, D2H], f16)
            bh_in = dram.tile([128, B], f16)
            bh_out = dram.tile([H, B], f16)

            # constants
            weE = const_pool.tile([128, D2H], f16)
            nc.sync.dma_start(weE[:], weE_d.ap())
            weH = const_pool.tile([128, KT_HH], f32)
            nc.sync.dma_start(weH[:], weH_d.ap())
            be_sb = const_pool.tile([1, 1], f32)
            nc.sync.dma_start(be_sb[:], be_d.ap())
            ones = const_pool.tile([128, 1], f32)
            nc.vector.memset(ones[:], 1.0)
            ones_r = const_pool.tile([128, 1], f16)
            nc.vector.memset(ones_r[:], 1.0)
            negk = const_pool.tile([128, 1], f32)
            nc.vector.memset(negk[:], -8.0)
            ones64 = const_pool.tile([1, B], f16)
            nc.vector.memset(ones64[:], 1.0)
            ident_h = const_pool.tile([128, 128], f16)
            make_identity(nc, ident_h[:])

            # ---- per-batch energy offset: off[b] = hidden[b] . We_h + be ----
            hTl_r = small.tile([128, BC, KT_HH], f32)
            nc.sync.dma_start(
                hTl_r[:], hTl_d.ap().rearrange("(c p) b -> p b c", p=128)
            )
            htmp = small.tile([128, BC, KT_HH], f32)
            nc.vector.tensor_tensor(
                out=htmp[:],
                in0=hTl_r[:],
                in1=weH[:][:, None, :].to_broadcast([128, BC, KT_HH]),
                op=OP.mult,
            )
            hpart = small.tile([128, BC], f32)
            nc.vector.tensor_reduce(
                out=hpart[:], in_=htmp[:], axis=mybir.AxisListType.X, op=OP.add
            )

            with tc.tile_pool(name="psum_off", bufs=1, space="PSUM") as psum_off:
                off_ps = psum_off.tile([1, BC], f32, space="PSUM")
                nc.tensor.matmul(
                    off_ps[:], lhsT=ones[:], rhs=hpart[:], start=True, stop=True
                )
                off_sb = small.tile([1, BC], f32)
                nc.scalar.activation(
                    off_sb[:], off_ps[:], AF.Identity, bias=be_sb[:], scale=1.0
                )
            off_rep = small.tile([128, BC], f32)
            nc.gpsimd.partition_broadcast(off_rep[:], off_sb[:])

            # ---- attention ----
            # Per-tile pipeline: energy dot (DVE) -> relu+exp (ACT) -> PE
            # accumulation of both U = sum_s w_s * es[s,:] and Z = sum_s w_s
            # (extra ones-column matmul), so each es tile is released as soon
            # as its 5 matmuls issue.
            with (
                tc.tile_pool(name="es", bufs=10) as es_pool,
                tc.tile_pool(name="scr", bufs=2) as scratch,
                tc.tile_pool(name="sm3", bufs=3) as sm3,
                tc.tile_pool(name="psU", bufs=1, space="PSUM") as psum_u,
                tc.tile_pool(name="psS", bufs=2, space="PSUM") as psum_s,
            ):
                for b in range(BC):
                    Eexp_b = sm3.tile([128, NT], f16, tag="E")
                    U = psum_u.tile([1, D2H], f32, space="PSUM", tag="U")
                    Uz = psum_s.tile([1, 1], f32, space="PSUM", tag="uz")
                    for t in range(NT):
                        et = es_pool.tile([128, D2H], f16, tag="es")
                        nc.sync.dma_start(
                            et[:], es_d.ap()[b, t * 128:(t + 1) * 128, :]
                        )
                        prod = scratch.tile([128, D2H], f16, tag="prod")
                        A1 = sm3.tile([128, 1], f32, tag="A")
                        nc.vector.scalar_tensor_tensor(
                            out=prod[:],
                            in0=et[:],
                            scalar=1.0,
                            in1=weE[:],
                            op0=OP.mult,
                            op1=OP.mult,
                            accum_out=A1[:],
                        )
                        R1 = sm3.tile([128, 1], f32, tag="R")
                        nc.scalar.activation(
                            R1[:], A1[:], AF.Relu,
                            bias=off_rep[:, b:b + 1], scale=1.0,
                        )
                        nc.scalar.activation(
                            Eexp_b[:, t:t + 1], R1[:], AF.Exp, bias=negk[:]
                        )
                        for j in range(D2H // 512):
                            nc.tensor.matmul(
                                U[:, j * 512:(j + 1) * 512],
                                lhsT=Eexp_b[:, t:t + 1],
                                rhs=et[:, j * 512:(j + 1) * 512],
                                start=(t == 0),
                                stop=(t == NT - 1),
                            )
                        nc.tensor.matmul(
                            Uz[:],
                            lhsT=Eexp_b[:, t:t + 1],
                            rhs=ones_r[:],
                            start=(t == 0),
                            stop=(t == NT - 1),
                        )
                    rZ_b = sm3.tile([1, 1], f32, tag="rz")
                    nc.vector.reciprocal(rZ_b[:], Uz[:])
                    stg = scratch.tile([1, D2H], f16, tag="stage")
                    nc.scalar.activation(
                        stg[:], U[:], AF.Copy, scale=rZ_b[:]
                    )
                    bxi = bx_in_a if b < BC // 2 else bx_in_b
                    nc.sync.dma_start(bxi[b % (BC // 2):b % (BC // 2) + 1, :], stg[:])
                    if b == BC // 2 - 1:
                        nc.gpsimd.collective_compute(
                            "AllGather", OP.bypass, replica_groups=groups,
                            ins=[bx_in_a.opt()], outs=[bx_out_a.opt()],
                        )


            # ---- gather second-half x_in across cores ----
            if stop_at < 2:
                raise _StopBuild(nc)
            nc.gpsimd.collective_compute(
                "AllGather", OP.bypass, replica_groups=groups,
                ins=[bx_in_b.opt()], outs=[bx_out_b.opt()],
            )

            # ---- LSTM (tensor-parallel over 128 h-rows) ----
            if stop_at < 3:
                raise _StopBuild(nc)
            with (
                tc.tile_pool(name="lstm", bufs=1) as lp,
                tc.tile_pool(name="psT", bufs=2, space="PSUM") as psum_t,
                tc.tile_pool(name="psG", bufs=4, space="PSUM") as psum_g,
            ):
                x_inT = lp.tile([128, KT_IH, B], f16)
                nc.sync.dma_start(
                    x_inT[:, D2H // 128:, :],
                    xembT_d.ap().rearrange("(c p) b -> p c b", p=128),
                )
                HB = B // 2
                for half, bxo in ((0, bx_out_a), (1, bx_out_b)):
                    x_full = lp.tile([HB, D2H], f16, tag=f"xf{half}")
                    nc.sync.dma_start(x_full[:], bxo[:])
                    for c in range(D2H // 128):
                        tp = psum_t.tile([128, HB], f16, space="PSUM", tag="tp")
                        nc.tensor.transpose(
                            tp[:], x_full[:, c * 128:(c + 1) * 128],
                            ident_h[:HB, :HB],
                        )
                        # gathered rows are core-major (4 local batches per
                        # core); scatter into global batch columns 8*core+lb
                        dst = x_inT[:, c, :].rearrange(
                            "p (i j) -> p i j", i=NCORES
                        )[:, :, half * (BC // 2):(half + 1) * (BC // 2)]
                        nc.vector.tensor_copy(dst, tp[:])

                hTf_sb = lp.tile([128, KT_HH, B], f16)
                nc.sync.dma_start(
                    hTf_sb[:], hTf_d.ap().rearrange("(t p) b -> p t b", p=128)
                )
                wih_sb = lp.tile([128, KT_IH, 512], f16)
                nc.sync.dma_start(
                    wih_sb[:], wih_d.ap().rearrange("t p g -> p t g")
                )
                whh_sb = lp.tile([128, KT_HH, 512], f16)
                nc.sync.dma_start(
                    whh_sb[:], whh_d.ap().rearrange("t p g -> p t g")
                )
                bias_sb = lp.tile([128, 4], f32)
                nc.sync.dma_start(bias_sb[:], bias_d.ap())
                cT_sb = lp.tile([128, B], f32)
                nc.sync.dma_start(cT_sb[:], cTk_d.ap())

                gps = []
                for c in range(4):
                    gp = psum_g.tile([128, B], f32, space="PSUM", tag="g")
                    gps.append(gp)
                    for kt in range(KT_IH):
                        nc.tensor.matmul(
                            gp[:],
                            lhsT=wih_sb[:, kt, c * 128:(c + 1) * 128],
                            rhs=x_inT[:, kt, :],
                            start=(kt == 0),
                            stop=False,
                        )
                    for kt in range(KT_HH):
                        nc.tensor.matmul(
                            gp[:],
                            lhsT=whh_sb[:, kt, c * 128:(c + 1) * 128],
                            rhs=hTf_sb[:, kt, :],
                            start=False,
                            stop=(kt == KT_HH - 1),
                        )

                # gates: order i, f, g, o
                acts = []
                for c, fn in enumerate([AF.Sigmoid, AF.Sigmoid, AF.Tanh, AF.Sigmoid]):
                    av = lp.tile([128, B], f32, tag=f"act{c}")
                    nc.scalar.activation(
                        av[:], gps[c][:], fn, bias=bias_sb[:, c:c + 1], scale=1.0
                    )
                    acts.append(av)
                i_s, f_s, g_t, o_s = acts

                t1 = lp.tile([128, B], f32)
                nc.vector.tensor_tensor(out=t1[:], in0=f_s[:], in1=cT_sb[:], op=OP.mult)
                t2 = lp.tile([128, B], f32)
                nc.vector.tensor_tensor(out=t2[:], in0=i_s[:], in1=g_t[:], op=OP.mult)
                cT_new = lp.tile([128, B], f32)
                nc.vector.tensor_tensor(out=cT_new[:], in0=t1[:], in1=t2[:], op=OP.add)
                th = lp.tile([128, B], f32)
                nc.scalar.activation(th[:], cT_new[:], AF.Tanh)
                hT_new = lp.tile([128, B], f32)
                nc.vector.tensor_tensor(out=hT_new[:], in0=o_s[:], in1=th[:], op=OP.mult)

                nc.sync.dma_start(hT_o.ap(), hT_new[:])
                nc.sync.dma_start(cT_o.ap(), cT_new[:])
                hb16 = lp.tile([128, B], f16)
                nc.vector.tensor_copy(hb16[:], hT_new[:])
                nc.sync.dma_start(bh_in[:], hb16[:])

            # ---- gather h_next across cores ----
            if stop_at < 4:
                raise _StopBuild(nc)
            nc.gpsimd.collective_compute(
                "AllGather", OP.bypass, replica_groups=groups,
                ins=[bh_in.opt()], outs=[bh_out.opt()],
            )

            # ---- vocab projection (tensor-parallel over V) ----
            if stop_at < 5:
                raise _StopBuild(nc)
            with (
                tc.tile_pool(name="proj", bufs=1) as pp,
                tc.tile_pool(name="wf", bufs=16) as wfp,
                tc.tile_pool(name="psP", bufs=2, space="PSUM") as psum_p,
            ):
                hTn_sb = pp.tile([128, KT_HH, B], f16)
                nc.sync.dma_start(
                    hTn_sb[:], bh_out[:].rearrange("(t p) b -> p t b", p=128)
                )
                bfr_sb = pp.tile([1, VC], f16)
                nc.sync.dma_start(bfr_sb[:], bfr_d.ap())
                preds_sb = pp.tile([B, VC], f32)

                for vt in range(NV):
                    wf_sb = wfp.tile([128, KT_HH, NTV], f16, tag="wf")
                    nc.sync.dma_start(
                        wf_sb[:], wf_d.ap()[vt].rearrange("t p g -> p t g")
                    )
                    pp_ps = psum_p.tile([B, NTV], f32, space="PSUM", tag="pp")
                    for kt in range(KT_HH):
                        nc.tensor.matmul(
                            pp_ps[:],
                            lhsT=hTn_sb[:, kt, :],
                            rhs=wf_sb[:, kt, :],
                            start=(kt == 0),
                            stop=False,
                        )
                    nc.tensor.matmul(
                        pp_ps[:],
                        lhsT=ones64[:],
                        rhs=bfr_sb[:, vt * NTV:(vt + 1) * NTV],
                        start=False,
                        stop=True,
                    )
                    nc.scalar.copy(preds_sb[:, vt * NTV:(vt + 1) * NTV], pp_ps[:])

                nc.sync.dma_start(preds_o.ap(), preds_sb[:])

    nc.compile()
    return nc


def _prep_inputs(encoder_states, x_tok, hidden, cell, emb, We, be,
                 W_ih, W_hh, b_ih, b_hh, Wf, bf):
    es = np.asarray(encoder_states, dtype=np.float32)
    hidden = np.asarray(hidden, dtype=np.float32)
    cell = np.asarray(cell, dtype=np.float32)
    We = np.asarray(We, dtype=np.float32)
    x_tok = np.asarray(x_tok)

    es16 = np.ascontiguousarray(es.astype(np.float16))
    hT = np.ascontiguousarray(hidden.T)                      # [H, B]
    hT16 = hT.astype(np.float16)
    cT = np.ascontiguousarray(cell.T)                        # [H, B]
    xembT = np.ascontiguousarray(
        np.asarray(emb, dtype=np.float32)[x_tok[:, 0]].T.astype(np.float16)
    )                                                        # [E, B]
    weE_rep = np.ascontiguousarray(
        np.broadcast_to(We[0, :D2H][None, :], (128, D2H))
    ).astype(np.float16)
    weH_pc = np.ascontiguousarray(We[0, D2H:].reshape(KT_HH, 128).T)  # [128, 8]
    be_in = np.asarray(be, dtype=np.float32).reshape(1, 1)

    W_ihT = np.ascontiguousarray(np.asarray(W_ih, dtype=np.float32).T)  # [DIN, 4H]
    W_hhT = np.ascontiguousarray(np.asarray(W_hh, dtype=np.float32).T)  # [H, 4H]
    bsum = np.asarray(b_ih, dtype=np.float32) + np.asarray(b_hh, dtype=np.float32)

    WfT = np.zeros((H, NCORES * VC), np.float32)
    WfT[:, :V] = np.asarray(Wf, dtype=np.float32).T
    bf_pad = np.zeros(NCORES * VC, np.float32)
    bf_pad[:V] = np.asarray(bf, dtype=np.float32)

    in_maps = []
    for k in range(NCORES):
        bs = slice(k * BC, (k + 1) * BC)
        hs = slice(k * 128, (k + 1) * 128)
        # gate columns for this core's h-slice: i, f, g, o
        gcols = np.concatenate([np.arange(g * H + k * 128, g * H + k * 128 + 128)
                                for g in range(4)])
        wih_k = np.ascontiguousarray(
            W_ihT[:, gcols].reshape(KT_IH, 128, 512).astype(np.float16)
        )
        whh_k = np.ascontiguousarray(
            W_hhT[:, gcols].reshape(KT_HH, 128, 512).astype(np.float16)
        )
        bias_k = np.ascontiguousarray(bsum[gcols].reshape(4, 128).T)  # [128, 4]
        wf_k = np.ascontiguousarray(
            WfT[:, k * VC:(k + 1) * VC]
            .reshape(KT_HH, 128, NV, NTV)
            .transpose(2, 0, 1, 3)
            .astype(np.float16)
        )
        in_maps.append({
            "es_k": es16[bs],
            "weE_rep": weE_rep,
            "weH_pc": weH_pc,
            "be_in": be_in,
            "hT_loc": np.ascontiguousarray(hT[:, bs]),
            "hT_full": hT16,
            "cT_k": np.ascontiguousarray(cT[hs, :]),
            "xembT": xembT,
            "wih_k": wih_k,
            "whh_k": whh_k,
            "bias_k": bias_k,
            "wf_k": wf_k,
            "bfr_k": bf_pad[None, k * VC:(k + 1) * VC].astype(np.float16),
        })
    return in_maps


def kernel(encoder_states, x_tok, hidden, cell, emb, We, be,
           W_ih, W_hh, b_ih, b_hh, Wf, bf, _trace=False):
    global _COMPILED
    if _COMPILED is None:
        _COMPILED = _build()
    nc = _COMPILED

    in_maps = _prep_inputs(encoder_states, x_tok, hidden, cell, emb, We, be,
                           W_ih, W_hh, b_ih, b_hh, Wf, bf)
    res = run_bass_kernel_spmd(
        nc, in_maps, core_ids=list(range(NCORES)), trace=_trace
    )
    outs = res.results

    preds = np.concatenate([outs[k]["preds_k"] for k in range(NCORES)], axis=1)
    preds = preds[:, :V][:, None, :]                      # [B, 1, V]
    h_next = np.concatenate(
        [outs[k]["hT_out"].T for k in range(NCORES)], axis=1
    )                                                     # [B, H]
    c_next = np.concatenate(
        [outs[k]["cT_out"].T for k in range(NCORES)], axis=1
    )
    if _trace:
        return (preds, h_next, c_next), res
    return preds, h_next, c_next
